# revision 6
# baseline (speedup 1.0000x reference)
"""Trainium2 Bass kernel for nn_Attention_75806172775136 (topk_masking).

Data-parallel over 8 NeuronCores: 8 samples per core, weights replicated.
Reference computes, per sample: qkv proj -> attn logits -> CLS-token top-138
mask -> masked softmax -> attn @ v -> out proj; returns (out, keep_mask,
attn_rt).

Per-core pipeline (layouts transposed so contraction rides partitions; host
pre-transposes inputs / post-transposes outputs):
  1. qkT GEMM (fp32r, N=394 sample-pairs): psum += Wqk @ xT, bias via K=1
     matmul; copied out twice: bf16 (attention) + f32 k-chunks/q0 (exact cls)
  2. v GEMM (fp32r) in natural [n, o] layout -> bf16
  3. cls scores: fp32 matmuls q0 . kT  (exact top-k selection vs reference)
  4. top-138 via vector.max + match_replace (exact-K, matches jax top_k)
  5. S^T = kT.T @ qT per (sample, head) in bf16; raw logits -> attn_rt
  6. e^T = exp(S^T) (no max-shift: |S| < 40); AV with lhsT = [v*keep | keep]
     -> out2T rows 0-63 numerator, row 64 denominator
  7. normalize: reciprocal -> gpsimd partition_broadcast -> DVE mul -> A^T
  8. out^T = Wp @ A^T + b (fp32r, N=394)
"""

import os
import sys

sys.path.insert(0, "/opt/trn_rl_repo")

import numpy as np

import concourse.bass as bass
import concourse.bacc as bacc
import concourse.mybir as mybir
from concourse.tile import TileContext
from concourse.bass_utils import run_bass_kernel_spmd

F32 = mybir.dt.float32
F32R = mybir.dt.float32r
BF16 = mybir.dt.bfloat16
AF = mybir.ActivationFunctionType
OP = mybir.AluOpType

NCORES = 8
B_CORE = 8
N = 197
C = 768
H = 12
HD = 64
KEEP = 138
NEG = -1.0e9

SPH = 4              # samples per half
NH = SPH * N         # 788
NPAIR = 2 * N        # 394


def _ts(i, s):
    return slice(i * s, (i + 1) * s)


def build_nc():
    nc = bacc.Bacc("TRN2", target_bir_lowering=False, debug=False)

    xT_d = nc.declare_dram_parameter("xT", [C, B_CORE * N], F32, isOutput=False)
    wqkvT_d = nc.declare_dram_parameter("wqkvT", [C, 3 * C], F32, isOutput=False)
    bqk_d = nc.declare_dram_parameter("bqk", [2 * C], F32, isOutput=False)
    bv_d = nc.declare_dram_parameter("bv", [C], F32, isOutput=False)
    wpT_d = nc.declare_dram_parameter("wpT", [C, C], F32, isOutput=False)
    bp_d = nc.declare_dram_parameter("bp", [C], F32, isOutput=False)
    ones_d = nc.declare_dram_parameter("ones", [NPAIR], F32, isOutput=False)

    outT_d = nc.declare_dram_parameter("outT", [C, B_CORE * N], F32, isOutput=True)
    keep_d = nc.declare_dram_parameter("keep", [B_CORE, N], F32, isOutput=True)
    # [b, h, k, q] — host transposes the last two dims
    art_d = nc.declare_dram_parameter("attn_rt_t", [B_CORE, H, N, N], F32, isOutput=True)

    with TileContext(nc) as tc, nc.allow_low_precision(reason="bf16/f32r compute by design"):
        with (
            tc.tile_pool(name="const", bufs=1) as cpool,
            tc.tile_pool(name="wq", bufs=1) as wpool,
            tc.tile_pool(name="xh", bufs=1) as xpool,
            tc.tile_pool(name="qk", bufs=1) as qkpool,
            tc.tile_pool(name="vv", bufs=4) as vpool,
            tc.tile_pool(name="v2", bufs=2) as v2pool,
            tc.tile_pool(name="att", bufs=4) as apool,
            tc.tile_pool(name="sc", bufs=1) as scpool,
            tc.tile_pool(name="aT", bufs=1) as atpool,
            tc.tile_pool(name="oT", bufs=2) as opool,
            tc.tile_pool(name="ps_mm", bufs=2, space="PSUM") as ps_mm,
            tc.tile_pool(name="ps_s", bufs=3, space="PSUM") as ps_s,
            tc.tile_pool(name="ps_av", bufs=2, space="PSUM") as ps_av,
            tc.tile_pool(name="ps_cls", bufs=1, space="PSUM") as ps_cls,
        ):
            # ---- weights / constants (persistent) ----
            w_sb = []
            for ci in range(6):
                t = wpool.tile([128, 3 * C], F32R, tag=f"wqkv{ci}")
                nc.sync.dma_start(out=t[:, :], in_=wqkvT_d[_ts(ci, 128), :].bitcast(F32R))
                w_sb.append(t)
            wp_sb = []
            for ci in range(6):
                t = wpool.tile([128, C], F32R, tag=f"wp{ci}")
                nc.sync.dma_start(out=t[:, :], in_=wpT_d[_ts(ci, 128), :].bitcast(F32R))
                wp_sb.append(t)
            bqk_row = cpool.tile([1, 2 * C], F32R)
            nc.sync.dma_start(out=bqk_row[:, :], in_=bqk_d[:].rearrange("(a c) -> a c", a=1).bitcast(F32R))
            bv_row = cpool.tile([1, C], F32R)
            nc.sync.dma_start(out=bv_row[:, :], in_=bv_d[:].rearrange("(a c) -> a c", a=1).bitcast(F32R))
            bp_row = cpool.tile([1, C], F32R)
            nc.sync.dma_start(out=bp_row[:, :], in_=bp_d[:].rearrange("(a c) -> a c", a=1).bitcast(F32R))
            ones_r = cpool.tile([1, NPAIR], F32R)
            nc.sync.dma_start(out=ones_r[:, :], in_=ones_d[:].rearrange("(a c) -> a c", a=1).bitcast(F32R))

            for half in range(2):
                b0 = half * SPH
                xh = xpool.tile([128, 6, NH], F32R, tag="xT")
                for ci in range(6):
                    nc.sync.dma_start(
                        out=xh[:, ci, :],
                        in_=xT_d[_ts(ci, 128), b0 * N : (b0 + SPH) * N].bitcast(F32R),
                    )

                # ---- qk GEMM: bf16 copy (attention) + f32 copy (cls) ----
                qkb = qkpool.tile([128, 12, NH], BF16, tag="qkb")
                ktf = qkpool.tile([128, 6, NH], F32, tag="ktf")
                q0f = qkpool.tile([128, 6, SPH], F32, tag="q0f")
                for oc in range(12):
                    for pr in range(2):
                        ps = ps_mm.tile([128, 512], F32, tag="mm")
                        for ci in range(6):
                            nc.tensor.matmul(
                                ps[:, :NPAIR],
                                lhsT=w_sb[ci][:, _ts(oc, 128)],
                                rhs=xh[:, ci, _ts(pr, NPAIR)],
                                start=(ci == 0),
                                stop=False,
                            )
                        nc.tensor.matmul(
                            ps[:, :NPAIR],
                            lhsT=bqk_row[:, _ts(oc, 128)],
                            rhs=ones_r[:, :],
                            start=False,
                            stop=True,
                        )
                        nc.scalar.copy(out=qkb[:, oc, _ts(pr, NPAIR)], in_=ps[:, :NPAIR])
                        if oc >= 6:  # k chunks also in f32 for cls scores
                            nc.vector.tensor_copy(out=ktf[:, oc - 6, _ts(pr, NPAIR)], in_=ps[:, :NPAIR])
                        else:  # q CLS columns for cls scores
                            for sb_ in range(2):
                                nc.vector.tensor_copy(
                                    out=q0f[:, oc, 2 * pr + sb_ : 2 * pr + sb_ + 1],
                                    in_=ps[:, sb_ * N : sb_ * N + 1],
                                )

                # ---- v GEMM (natural [n, o] layout) -> bf16 ----
                v_sb = []
                for bl in range(SPH):
                    vt = vpool.tile([128, 2, C], BF16, tag="v")
                    for chk, (r0, rn) in enumerate(((0, 128), (128, 69))):
                        for c0, cn in ((0, 512), (512, 256)):
                            ps = ps_mm.tile([128, 512], F32, tag="mm")
                            for ci in range(6):
                                nc.tensor.matmul(
                                    ps[:rn, :cn],
                                    lhsT=xh[:, ci, bl * N + r0 : bl * N + r0 + rn],
                                    rhs=w_sb[ci][:, 2 * C + c0 : 2 * C + c0 + cn],
                                    start=(ci == 0),
                                    stop=False,
                                )
                            nc.tensor.matmul(
                                ps[:rn, :cn],
                                lhsT=ones_r[:, :rn],
                                rhs=bv_row[:, c0 : c0 + cn],
                                start=False,
                                stop=True,
                            )
                            nc.scalar.copy(out=vt[:rn, chk, c0 : c0 + cn], in_=ps[:rn, :cn])
                    v_sb.append(vt)

                # ---- cls scores (fp32 exact) ----
                sc4 = scpool.tile([SPH, N], F32, tag="sc4")
                for bl in range(SPH):
                    ps = ps_cls.tile([1, N], F32, tag="cls")
                    for ci in range(6):
                        nc.tensor.matmul(
                            ps[:, :],
                            lhsT=q0f[:, ci, bl : bl + 1],
                            rhs=ktf[:, ci, bl * N : bl * N + N],
                            start=(ci == 0),
                            stop=(ci == 5),
                        )
                    row = scpool.tile([1, N], F32, tag=f"scrow{bl}")
                    nc.vector.tensor_copy(out=row[:, :], in_=ps[:, :])
                    nc.sync.dma_start(out=sc4[bl : bl + 1, :], in_=row[:, :])

                # ---- top-K scan ----
                work = scpool.tile([SPH, N], F32, tag="work")
                nc.vector.tensor_copy(out=work[:, :], in_=sc4[:, :])
                nc.vector.memset(work[:, 0:1], -2.0e9)
                m8 = scpool.tile([SPH, 8], F32, tag="m8")
                left = KEEP
                while left > 0:
                    nc.vector.max(out=m8[:, :], in_=work[:, :])
                    if left < 8:
                        nc.vector.memset(m8[:, left:], NEG)
                    nc.vector.match_replace(
                        out=work[:, :], in_to_replace=m8[:, :],
                        in_values=work[:, :], imm_value=NEG,
                    )
                    left -= 8
                keep4 = scpool.tile([SPH, N], F32, tag="keep4")
                nc.vector.tensor_tensor(out=keep4[:, :], in0=work[:, :], in1=sc4[:, :], op=OP.not_equal)
                nc.sync.dma_start(out=keep_d[b0 : b0 + SPH, :], in_=keep4[:, :])

                # keepT via DMA partition-scatter
                kT1 = scpool.tile([128, SPH], F32, tag="kT1")
                kT2 = scpool.tile([128, SPH], F32, tag="kT2")
                for bl in range(SPH):
                    nc.sync.dma_start(out=kT1[:, bl : bl + 1], in_=keep4[bl : bl + 1, 0:128])
                    nc.sync.dma_start(out=kT2[0:69, bl : bl + 1], in_=keep4[bl : bl + 1, 128:197])

                # ---- V2 = [v*keep | keep] bf16 ----
                v2_sb = []
                for bl in range(SPH):
                    v2 = v2pool.tile([128, 2, H, HD + 1], BF16, tag="v2")
                    for chk, (kTc, rn) in enumerate(((kT1, 128), (kT2, 69))):
                        nc.vector.tensor_scalar(
                            out=v2[:rn, chk, :, 0:HD],
                            in0=v_sb[bl][:rn, chk, :].rearrange("p (h d) -> p h d", h=H),
                            scalar1=kTc[:rn, bl : bl + 1],
                            scalar2=None,
                            op0=OP.mult,
                        )
                        nc.vector.tensor_copy(
                            out=v2[:rn, chk, :, HD],
                            in_=kTc[:rn, bl : bl + 1].to_broadcast([rn, H]),
                        )
                    v2_sb.append(v2)

                # ---- attention per (sample, head) ----
                aT = atpool.tile([128, 6, NH], F32R, tag="aT")
                for bl in range(SPH):
                    b = b0 + bl
                    for h in range(H):
                        oc, po = h // 2, (h % 2) * HD
                        e_t = []
                        for chk, (r0, rn) in enumerate(((0, 128), (128, 69))):
                            ps = ps_s.tile([128, N], F32, tag="sT")
                            nc.tensor.matmul(
                                ps[:rn, :],
                                lhsT=qkb[po : po + HD, 6 + oc, bl * N + r0 : bl * N + r0 + rn],
                                rhs=qkb[po : po + HD, oc, bl * N : bl * N + N],
                                start=True,
                                stop=True,
                            )
                            st = apool.tile([128, N], F32, tag="sT_sb")
                            eng = nc.scalar if chk == 0 else nc.vector
                            if eng is nc.scalar:
                                nc.scalar.copy(out=st[:rn, :], in_=ps[:rn, :])
                            else:
                                nc.vector.tensor_copy(out=st[:rn, :], in_=ps[:rn, :])
                            nc.sync.dma_start(out=art_d[b, h, r0 : r0 + rn, :], in_=st[:rn, :])
                            et = apool.tile([128, N], BF16, tag="eT")
                            nc.scalar.activation(out=et[:rn, :], in_=ps[:rn, :], func=AF.Exp)
                            e_t.append((et, rn))

                        ps_o = ps_av.tile([HD + 1, N], F32, tag="av")
                        for chk, (et, rn) in enumerate(e_t):
                            nc.tensor.matmul(
                                ps_o[:, :],
                                lhsT=v2_sb[bl][:rn, chk, h, :],
                                rhs=et[:rn, :],
                                start=(chk == 0),
                                stop=(chk == 1),
                            )
                        rr = apool.tile([1, N], F32, tag="rr")
                        nc.vector.reciprocal(out=rr[0:1, :], in_=ps_o[HD : HD + 1, :])
                        bc = apool.tile([HD, N], F32, tag="bc")
                        nc.gpsimd.partition_broadcast(bc[:, :], rr[0:1, :])
                        nc.vector.tensor_tensor(
                            out=aT[po : po + HD, oc, bl * N : bl * N + N],
                            in0=ps_o[0:HD, :],
                            in1=bc[:, :],
                            op=OP.mult,
                        )

                # ---- proj ----
                for pr in range(2):
                    for oc in range(6):
                        ps = ps_mm.tile([128, 512], F32, tag="mm")
                        for ci in range(6):
                            nc.tensor.matmul(
                                ps[:, :NPAIR],
                                lhsT=wp_sb[ci][:, _ts(oc, 128)],
                                rhs=aT[:, ci, _ts(pr, NPAIR)],
                                start=(ci == 0),
                                stop=False,
                            )
                        nc.tensor.matmul(
                            ps[:, :NPAIR],
                            lhsT=bp_row[:, _ts(oc, 128)],
                            rhs=ones_r[:, :],
                            start=False,
                            stop=True,
                        )
                        ot = opool.tile([128, NPAIR], F32, tag="oT")
                        nc.scalar.copy(out=ot[:, :], in_=ps[:, :NPAIR])
                        nc.sync.dma_start(
                            out=outT_d[_ts(oc, 128), (b0 + 2 * pr) * N : (b0 + 2 * pr + 2) * N],
                            in_=ot[:, :],
                        )
    nc.finalize()
    return nc


_NC_CACHE = None
LAST_RESULT = None


def kernel(x, qkv_w, qkv_b, proj_w, proj_b, num_keep_node):
    global _NC_CACHE
    assert int(num_keep_node) == KEEP
    x = np.asarray(x, np.float32)
    scale = float(HD) ** -0.5

    wqkvT = np.ascontiguousarray(np.asarray(qkv_w, np.float32).T)  # [768, 2304]
    wqkvT[:, :C] = wqkvT[:, :C] * scale
    bqk = np.asarray(qkv_b, np.float32)[: 2 * C].copy()
    bqk[:C] *= scale
    bv = np.ascontiguousarray(np.asarray(qkv_b, np.float32)[2 * C :])
    wpT = np.ascontiguousarray(np.asarray(proj_w, np.float32).T)
    bp = np.ascontiguousarray(np.asarray(proj_b, np.float32))
    ones = np.ones(NPAIR, np.float32)

    if _NC_CACHE is None:
        _NC_CACHE = build_nc()
    nc = _NC_CACHE

    in_maps = []
    for c in range(NCORES):
        xs = x[c * B_CORE : (c + 1) * B_CORE].reshape(B_CORE * N, C)
        in_maps.append(
            {
                "xT": np.ascontiguousarray(xs.T),
                "wqkvT": wqkvT,
                "bqk": bqk,
                "bv": bv,
                "wpT": wpT,
                "bp": bp,
                "ones": ones,
            }
        )
    global LAST_RESULT
    LAST_RESULT = run_bass_kernel_spmd(
        nc,
        in_maps,
        core_ids=list(range(NCORES)),
        trace=bool(os.environ.get("KTRACE")),
        tmpdir=os.environ.get("KTRACE_DIR") or None,
    )
    res = LAST_RESULT.results

    outs, keeps, arts = [], [], []
    for r in res:
        outs.append(np.ascontiguousarray(r["outT"].reshape(C, B_CORE, N).transpose(1, 2, 0)))
        keeps.append(r["keep"][:, :, None])
        arts.append(np.ascontiguousarray(r["attn_rt_t"].transpose(0, 1, 3, 2)))
    return (
        np.concatenate(outs, 0),
        np.concatenate(keeps, 0),
        np.concatenate(arts, 0),
    )
